# revision 14
# baseline (speedup 1.0000x reference)
"""Bass/Trainium2 kernel for GQA attention prefill (nn_Attention).

Reference computation (per core, tensor-parallel over 8 NeuronCores):
  q = x @ wq ; k = x @ wk ; v = x @ wv      (core i owns 4 Q heads + 1 KV head)
  q,k = rope(q), rope(k)                     (interleaved-pair RoPE)
  attn = causal_softmax(q k^T / sqrt(hd)) v  (head-local under GQA)
  out  = attn_flat @ wo                      (column-parallel: AllGather the
                                              small attn output, each core
                                              computes a 512-col slice of out)

Host-side tricks:
  - x is passed pre-transposed (xT, [D, S]) so both projection orientations
    need no on-device transpose.
  - wq/wk columns are permuted per head (even pair-indices first) so RoPE
    becomes block arithmetic on [0:64]/[64:128] partition halves. Scores are
    invariant to a shared permutation of q/k features.
  - cos/sin passed transposed ([64, S]); causal masks for diagonal blocks
    passed precomputed.
  - All matmul operands in bf16 (PSUM accumulates f32); rel-err gate 2e-2.

Scheduling notes:
  - Compute instructions (TT/ACT) carry at most ~8 hardware sync-wait slots;
    SBUF address reuse across pools makes the first tenant-writer inherit the
    previous tenant's DMA-queue wait set. So the attention-phase pools are
    kept address-disjoint from the projection streaming pools (both fit),
    and big tensors are loaded with ~4 DMAs each to bound queue fan-out.
"""

import sys

sys.path.insert(0, "/opt/trn_rl_repo")

import numpy as np
import ml_dtypes

import concourse.bass as bass
import concourse.mybir as mybir
import concourse.tile as tile
from concourse import bacc
from concourse.masks import make_identity

BF16 = mybir.dt.bfloat16
F32 = mybir.dt.float32
P = 128
HD = 128  # head dim

N_CORES = 8
B, S_FULL, D_FULL = 1, 2048, 4096
NH, NKV = 32, 8
HLOC = NH // N_CORES  # 4 q heads per core


def build_nc(n_cores=8, S=2048, D=4096, hloc=4, QT=512):
    """Build the SPMD Bass graph (same graph on every core)."""
    KC = D // P            # contraction chunks for the projections
    DQ = hloc * HD         # local q width (512)
    WCOLS = DQ + 2 * HD    # wq|wk|wv concatenated
    NKB = S // P           # key blocks
    NQT = S // QT          # q tiles
    DIAG = QT // P         # diagonal (partially masked) blocks per q tile
    OUTW = D // n_cores    # output column slice per core
    ND = n_cores * DQ      # full attention dim (NH*HD)
    NDC = ND // P          # allgathered chunks
    NSB = S // P           # output row blocks
    XG = max(1, KC // 4)   # kc chunks per xT streaming piece
    NXG = KC // XG
    scale = 1.0 / float(np.sqrt(HD))
    Exp = mybir.ActivationFunctionType.Exp

    nc = bacc.Bacc()
    xT = nc.declare_dram_parameter("xT", [D, S], BF16, isOutput=False)
    wqkv = nc.declare_dram_parameter("wqkv", [D, WCOLS], BF16, isOutput=False)
    wo = nc.declare_dram_parameter("wo", [ND, OUTW], BF16, isOutput=False)
    cosT = nc.declare_dram_parameter("cosT", [HD // 2, S], F32, isOutput=False)
    sinT = nc.declare_dram_parameter("sinT", [HD // 2, S], F32, isOutput=False)
    maskT = nc.declare_dram_parameter("maskT", [P, DIAG, QT], BF16, isOutput=False)
    out = nc.declare_dram_parameter("out", [S, OUTW], F32, isOutput=True)

    xT_r = xT.rearrange("(kc p) s -> p kc s", p=P)
    wqkv_r = wqkv.rearrange("(kc p) w -> p kc w", p=P)
    wo_r = wo.rearrange("(kc p) n -> p kc n", p=P)

    with tile.TileContext(nc) as tc:
        with (
            tc.tile_pool(name="dramp", bufs=1, space="DRAM") as dramp,
            tc.tile_pool(name="constp", bufs=1) as constp,
            tc.tile_pool(name="outp", bufs=3) as outp,
        ):
            ones_sb = constp.tile([P, 1], BF16, name="ones_sb")
            nc.vector.memset(ones_sb, 1.0)
            ones_row = constp.tile([1, P], F32, name="ones_row")
            nc.vector.memset(ones_row, 1.0)
            ident_sb = constp.tile([P, P], BF16, name="ident_sb")
            make_identity(nc, ident_sb)
            cos_sb = constp.tile([HD // 2, S], F32, name="cos_sb")
            sin_sb = constp.tile([HD // 2, S], F32, name="sin_sb")
            nc.sync.dma_start(cos_sb, cosT[:, :])
            nc.sync.dma_start(sin_sb, sinT[:, :])
            mask_sb = constp.tile([P, DIAG, QT], BF16, name="mask_sb")
            nc.sync.dma_start(mask_sb, maskT[:, :, :])

            in_b = dramp.tile([DQ, S], BF16, name="in_b")
            ag_out = dramp.tile([ND, S], BF16, name="ag_out", addr_space="Shared")

            with (
                tc.tile_pool(name="qkvp", bufs=1) as qkvp,
                tc.tile_pool(name="attp", bufs=1) as attp,
                tc.tile_pool(name="probsp", bufs=4) as probsp,
                tc.tile_pool(name="smallp", bufs=4) as smallp,
                tc.tile_pool(name="bcp", bufs=2) as bcp,
            ):
                q_sb = [
                    qkvp.tile([P, S], BF16, name=f"q_sb{h}") for h in range(hloc)
                ]
                k_sb = qkvp.tile([P, S], BF16, name="k_sb")
                vT_sb = qkvp.tile([P, S], BF16, name="vT_sb")
                v_sb = qkvp.tile([P, NKB, HD], BF16, name="v_sb")
                attnT_sb = [
                    attp.tile([P, S], BF16, name=f"attnT_sb{h}")
                    for h in range(hloc)
                ]

                # ---------------- phase 1: projections + rope ----------------
                with (
                    tc.tile_pool(name="wqkvp", bufs=1) as wqkvp,
                    tc.tile_pool(name="xtp", bufs=NXG + 2) as xtp,
                    tc.tile_pool(name="ptmp", bufs=2) as ptmp,
                    tc.tile_pool(name="psA", bufs=3, space="PSUM") as psA,
                    tc.tile_pool(name="psT", bufs=2, space="PSUM") as psT,
                ):
                    wqkv_sb = wqkvp.tile([P, KC, WCOLS], BF16, name="wqkv_sb")
                    for g in range(NXG):
                        nc.sync.dma_start(
                            wqkv_sb[:, g * XG : (g + 1) * XG, :],
                            wqkv_r[:, g * XG : (g + 1) * XG, :],
                        )

                    H2 = HD // 2
                    for st in range(NQT):
                        sl = slice(st * QT, (st + 1) * QT)
                        xg = []
                        for g in range(NXG):
                            t = xtp.tile(
                                [P, XG, QT], BF16, tag="xt", name=f"xt{st}_{g}"
                            )
                            nc.sync.dma_start(t, xT_r[:, g * XG : (g + 1) * XG, sl])
                            xg.append(t)
                        for mb in range(hloc + 2):
                            ps = psA.tile([P, QT], F32, tag="ps", name=f"ps{st}_{mb}")
                            for kc in range(KC):
                                nc.tensor.matmul(
                                    ps,
                                    wqkv_sb[:, kc, mb * P : (mb + 1) * P],
                                    xg[kc // XG][:, kc % XG, :],
                                    start=(kc == 0),
                                    stop=(kc == KC - 1),
                                )
                            if mb <= hloc:  # q heads and k: rope
                                dst = q_sb[mb] if mb < hloc else k_sb
                                csl = cos_sb[:, sl]
                                ssl = sin_sb[:, sl]
                                t1 = ptmp.tile([H2, QT], F32, tag="t1", name=f"t1_{st}_{mb}")
                                t2 = ptmp.tile([H2, QT], F32, tag="t2", name=f"t2_{st}_{mb}")
                                nc.vector.tensor_mul(t1, ps[H2:P, :], ssl)
                                nc.vector.tensor_mul(t2, ps[0:H2, :], csl)
                                nc.vector.tensor_sub(dst[0:H2, sl], t2, t1)
                                t3 = ptmp.tile([H2, QT], F32, tag="t3", name=f"t3_{st}_{mb}")
                                t4 = ptmp.tile([H2, QT], F32, tag="t4", name=f"t4_{st}_{mb}")
                                nc.vector.tensor_mul(t3, ps[0:H2, :], ssl)
                                nc.vector.tensor_mul(t4, ps[H2:P, :], csl)
                                nc.vector.tensor_add(dst[H2:P, sl], t3, t4)
                            else:  # v: keep transposed copy, fix up below
                                nc.vector.tensor_copy(vT_sb[:, sl], ps)

                    # transpose vT -> v (natural [s, hd] layout for PV lhsT)
                    for kb in range(NKB):
                        pt = psT.tile([P, P], BF16, tag="pt", name=f"pt{kb}")
                        nc.tensor.transpose(
                            pt, vT_sb[:, kb * P : (kb + 1) * P], ident_sb
                        )
                        nc.vector.tensor_copy(v_sb[:, kb, :], pt)

                # ---------------- phase 2: attention ----------------
                with (
                    tc.tile_pool(name="psS", bufs=3, space="PSUM") as psS,
                    tc.tile_pool(name="psPA", bufs=2, space="PSUM") as psPA,
                    tc.tile_pool(name="psD", bufs=1, space="PSUM") as psD,
                    tc.tile_pool(name="psB", bufs=1, space="PSUM") as psB,
                ):
                    for h in range(hloc):
                        for qt in range(NQT):
                            qsl = slice(qt * QT, (qt + 1) * QT)
                            nkb = (qt + 1) * DIAG
                            pa = psPA.tile([P, QT], F32, tag="pa", name=f"pa{h}_{qt}")
                            pd = psD.tile([1, QT], F32, tag="pd", name=f"pd{h}_{qt}")
                            for kb in range(nkb):
                                ps = psS.tile(
                                    [P, QT], F32, tag="pssc", name=f"sc{h}_{qt}_{kb}"
                                )
                                nc.tensor.matmul(
                                    ps,
                                    k_sb[:, kb * P : (kb + 1) * P],
                                    q_sb[h][:, qsl],
                                    start=True,
                                    stop=True,
                                )
                                pt = probsp.tile(
                                    [P, QT], BF16, tag="probs", name=f"pb{h}_{qt}_{kb}"
                                )
                                nc.scalar.activation(pt, ps, Exp, scale=scale)
                                o = kb - qt * DIAG
                                if o >= 0:
                                    nc.vector.tensor_mul(pt, pt, mask_sb[:, o, :])
                                nc.tensor.matmul(
                                    pa,
                                    v_sb[:, kb, :],
                                    pt,
                                    start=(kb == 0),
                                    stop=(kb == nkb - 1),
                                )
                                nc.tensor.matmul(
                                    pd,
                                    ones_sb,
                                    pt,
                                    start=(kb == 0),
                                    stop=(kb == nkb - 1),
                                )
                            den = smallp.tile([1, QT], F32, tag="den", name=f"dn{h}_{qt}")
                            nc.vector.tensor_copy(den, pd)
                            rec = smallp.tile([1, QT], F32, tag="rec", name=f"rc{h}_{qt}")
                            nc.vector.reciprocal(rec, den)
                            # broadcast rec across partitions: ones ⊗ rec
                            rbp = psB.tile([P, QT], F32, tag="rbp", name=f"rp{h}_{qt}")
                            nc.tensor.matmul(rbp, ones_row, rec, start=True, stop=True)
                            rbc = bcp.tile([P, QT], F32, tag="rbc", name=f"rb{h}_{qt}")
                            nc.vector.tensor_copy(rbc, rbp)
                            nc.vector.tensor_mul(attnT_sb[h][:, qsl], pa, rbc)

                # flush local attn slice to DRAM bounce for the collective
                for h in range(hloc):
                    nc.sync.dma_start(in_b[h * P : (h + 1) * P, :], attnT_sb[h])

            # ---------------- phase 3: allgather + output projection ----------
            nc.gpsimd.collective_compute(
                "AllGather",
                mybir.AluOpType.bypass,
                replica_groups=[list(range(n_cores))],
                ins=[in_b.opt()],
                outs=[ag_out.opt()],
            )

            with (
                tc.tile_pool(name="agp", bufs=NDC) as agp,
                tc.tile_pool(name="wop", bufs=1) as wop,
                tc.tile_pool(name="psO", bufs=2, space="PSUM") as psO,
            ):
                wo_sb = wop.tile([P, NDC, OUTW], BF16, name="wo_sb")
                WG = max(1, NDC // 4)
                for g in range(NDC // WG):
                    nc.sync.dma_start(
                        wo_sb[:, g * WG : (g + 1) * WG, :],
                        wo_r[:, g * WG : (g + 1) * WG, :],
                    )
                ag_sb = []
                for kc in range(NDC):
                    t = agp.tile([P, S], BF16, tag="agt", name=f"ag{kc}")
                    nc.sync.dma_start(t, ag_out[kc * P : (kc + 1) * P, :])
                    ag_sb.append(t)
                for sb in range(NSB):
                    po = psO.tile([P, OUTW], F32, tag="po", name=f"po{sb}")
                    for kc in range(NDC):
                        nc.tensor.matmul(
                            po,
                            ag_sb[kc][:, sb * P : (sb + 1) * P],
                            wo_sb[:, kc, :],
                            start=(kc == 0),
                            stop=(kc == NDC - 1),
                        )
                    ot = outp.tile([P, OUTW], F32, tag="ot", name=f"ot{sb}")
                    nc.vector.tensor_copy(ot, po)
                    nc.sync.dma_start(out[sb * P : (sb + 1) * P, :], ot)
    nc.finalize()
    return nc


def _rope_perm(width):
    """Per-head column permutation putting even pair-indices first."""
    blocks = []
    for h in range(width // HD):
        base = h * HD
        blocks.append(base + np.r_[0:HD:2, 1:HD:2])
    return np.concatenate(blocks)


def make_in_maps(x, wq, wk, wv, wo, cos, sin, n_cores=8, QT=512):
    """Shard + preprocess full inputs into per-core input maps."""
    bf16 = ml_dtypes.bfloat16
    S, D = x.shape[1], x.shape[2]
    nh = wq.shape[1] // HD
    hloc = nh // n_cores
    DQ = hloc * HD
    KVW = HD  # one kv head per core
    OUTW = D // n_cores
    DIAG = QT // P

    xT = np.ascontiguousarray(np.asarray(x[0]).T).astype(bf16)
    wq_p = np.asarray(wq)[:, _rope_perm(wq.shape[1])]
    wk_p = np.asarray(wk)[:, _rope_perm(wk.shape[1])]
    wv = np.asarray(wv)
    wo = np.asarray(wo)
    cosT = np.ascontiguousarray(np.asarray(cos).T).astype(np.float32)
    sinT = np.ascontiguousarray(np.asarray(sin).T).astype(np.float32)

    kk = np.arange(P)[:, None, None]
    oo = np.arange(DIAG)[None, :, None]
    qq = np.arange(QT)[None, None, :]
    maskT = ((kk + P * oo) <= qq).astype(bf16)

    in_maps = []
    for i in range(n_cores):
        wqkv_i = np.concatenate(
            [
                wq_p[:, i * DQ : (i + 1) * DQ],
                wk_p[:, i * KVW : (i + 1) * KVW],
                wv[:, i * KVW : (i + 1) * KVW],
            ],
            axis=1,
        ).astype(bf16)
        wo_i = np.ascontiguousarray(wo[:, i * OUTW : (i + 1) * OUTW]).astype(bf16)
        in_maps.append(
            {
                "xT": xT,
                "wqkv": np.ascontiguousarray(wqkv_i),
                "wo": wo_i,
                "cosT": cosT,
                "sinT": sinT,
                "maskT": np.ascontiguousarray(maskT),
            }
        )
    return in_maps


_CACHED_NC = None


def kernel(x, wq, wk, wv, wo, cos, sin):
    global _CACHED_NC
    from concourse.bass_utils import run_bass_kernel_spmd

    n_cores = N_CORES
    if _CACHED_NC is None:
        _CACHED_NC = build_nc(
            n_cores=n_cores, S=x.shape[1], D=x.shape[2], hloc=HLOC, QT=512
        )
    in_maps = make_in_maps(x, wq, wk, wv, wo, cos, sin, n_cores=n_cores, QT=512)
    res = run_bass_kernel_spmd(_CACHED_NC, in_maps, core_ids=list(range(n_cores)))
    outs = [np.asarray(res.results[i]["out"]) for i in range(n_cores)]
    full = np.concatenate(outs, axis=1).astype(np.float32)
    return full.reshape(1, x.shape[1], x.shape[2])


if __name__ == "__main__":
    nc = build_nc()
    print("graph built ok")


# revision 21
# speedup vs baseline: 1.2679x; 1.2679x over previous
"""Bass/Trainium2 kernel for GQA attention prefill (nn_Attention).

Reference computation (per core, tensor-parallel over 8 NeuronCores):
  q = x @ wq ; k = x @ wk ; v = x @ wv      (core i owns 4 Q heads + 1 KV head)
  q,k = rope(q), rope(k)                     (interleaved-pair RoPE)
  attn = causal_softmax(q k^T / sqrt(hd)) v  (head-local under GQA)
  out  = attn_flat @ wo                      (column-parallel: AllGather the
                                              small attn output, each core
                                              computes a 512-col slice of out)

Host-side tricks:
  - x is passed pre-transposed (xT, [D, S]) so both projection orientations
    need no on-device transpose.
  - wq/wk columns are permuted per head (even pair-indices first) so RoPE
    becomes block arithmetic on [0:64]/[64:128] partition halves. Scores are
    invariant to a shared permutation of q/k features.
  - cos/sin passed transposed ([64, S]); causal masks for diagonal blocks
    passed precomputed.
  - All matmul operands in bf16 (PSUM accumulates f32); rel-err gate 2e-2.

Scheduling notes:
  - Compute instructions (TT/ACT) carry at most ~8 hardware sync-wait slots;
    SBUF address reuse across pools makes the first tenant-writer inherit the
    previous tenant's DMA-queue wait set. So the attention-phase pools are
    kept address-disjoint from the projection streaming pools (both fit),
    and big tensors are loaded with ~4 DMAs each to bound queue fan-out.
"""

import sys

sys.path.insert(0, "/opt/trn_rl_repo")

import numpy as np
import ml_dtypes

import concourse.bass as bass
import concourse.mybir as mybir
import concourse.tile as tile
from concourse import bacc
from concourse.masks import make_identity

BF16 = mybir.dt.bfloat16
F32 = mybir.dt.float32
P = 128
HD = 128  # head dim

N_CORES = 8
B, S_FULL, D_FULL = 1, 2048, 4096
NH, NKV = 32, 8
HLOC = NH // N_CORES  # 4 q heads per core


def build_nc(n_cores=8, S=2048, D=4096, hloc=4, QT=512):
    """Build the SPMD Bass graph (same graph on every core)."""
    KC = D // P            # contraction chunks for the projections
    DQ = hloc * HD         # local q width (512)
    WCOLS = DQ + 2 * HD    # wq|wk|wv concatenated
    NKB = S // P           # key blocks
    NQT = S // QT          # q tiles
    DIAG = QT // P         # diagonal (partially masked) blocks per q tile
    OUTW = D // n_cores    # output column slice per core
    ND = n_cores * DQ      # full attention dim (NH*HD)
    NDC = ND // P          # allgathered chunks
    NSB = S // P           # output row blocks
    XG = max(1, KC // 4)   # kc chunks per xT streaming piece
    NXG = KC // XG
    scale = 1.0 / float(np.sqrt(HD))
    Exp = mybir.ActivationFunctionType.Exp

    nc = bacc.Bacc()
    xT = nc.declare_dram_parameter("xT", [D, S], BF16, isOutput=False)
    wqkv = nc.declare_dram_parameter("wqkv", [D, WCOLS], BF16, isOutput=False)
    wo = nc.declare_dram_parameter("wo", [ND, OUTW], BF16, isOutput=False)
    cosT = nc.declare_dram_parameter("cosT", [HD // 2, S], F32, isOutput=False)
    sinT = nc.declare_dram_parameter("sinT", [HD // 2, S], F32, isOutput=False)
    maskT = nc.declare_dram_parameter("maskT", [P, DIAG, QT], BF16, isOutput=False)
    out = nc.declare_dram_parameter("out", [S, OUTW], F32, isOutput=True)

    xT_r = xT.rearrange("(kc p) s -> p kc s", p=P)
    wqkv_r = wqkv.rearrange("(kc p) w -> p kc w", p=P)
    wo_r = wo.rearrange("(kc p) n -> p kc n", p=P)

    with tile.TileContext(nc) as tc:
        with (
            tc.tile_pool(name="dramp", bufs=1, space="DRAM") as dramp,
            tc.tile_pool(name="constp", bufs=1) as constp,
            tc.tile_pool(name="outp", bufs=3) as outp,
        ):
            ones_bc = constp.tile([P, P], BF16, name="ones_bc")
            nc.vector.memset(ones_bc, 1.0)
            ident_sb = constp.tile([P, P], BF16, name="ident_sb")
            make_identity(nc, ident_sb)
            cos_sb = constp.tile([HD // 2, S], F32, name="cos_sb")
            sin_sb = constp.tile([HD // 2, S], F32, name="sin_sb")
            nc.sync.dma_start(cos_sb, cosT[:, :])
            nc.sync.dma_start(sin_sb, sinT[:, :])
            mask_sb = constp.tile([P, DIAG, QT], BF16, name="mask_sb")
            nc.sync.dma_start(mask_sb, maskT[:, :, :])

            in_bs = [
                dramp.tile([P, S], BF16, name=f"in_b{h}") for h in range(hloc)
            ]
            ag_outs = [
                dramp.tile([n_cores * P, S], BF16, name=f"agd{h}", addr_space="Shared")
                for h in range(hloc)
            ]

            with (
                tc.tile_pool(name="qkvp", bufs=1) as qkvp,
                tc.tile_pool(name="attp", bufs=1) as attp,
                tc.tile_pool(name="probsp", bufs=4) as probsp,
                tc.tile_pool(name="bcp", bufs=2) as bcp,
            ):
                q_sb = [
                    qkvp.tile([P, S], BF16, name=f"q_sb{h}") for h in range(hloc)
                ]
                k_sb = qkvp.tile([P, S], BF16, name="k_sb")
                vT_sb = qkvp.tile([P, S], BF16, name="vT_sb")
                v_sb = qkvp.tile([P, NKB, HD], BF16, name="v_sb")
                attnT_sb = [
                    attp.tile([P, S], BF16, name=f"attnT_sb{h}")
                    for h in range(hloc)
                ]

                # ---------------- phase 1: projections + rope ----------------
                with (
                    tc.tile_pool(name="wqkvp", bufs=1) as wqkvp,
                    tc.tile_pool(name="xtp", bufs=NXG + 2) as xtp,
                    tc.tile_pool(name="ptmp", bufs=2) as ptmp,
                    tc.tile_pool(name="psA", bufs=3, space="PSUM") as psA,
                    tc.tile_pool(name="psT", bufs=2, space="PSUM") as psT,
                ):
                    wqkv_sb = wqkvp.tile([P, KC, WCOLS], BF16, name="wqkv_sb")
                    WQG = max(1, KC // 8)
                    for g in range(KC // WQG):
                        nc.sync.dma_start(
                            wqkv_sb[:, g * WQG : (g + 1) * WQG, :],
                            wqkv_r[:, g * WQG : (g + 1) * WQG, :],
                        )

                    H2 = HD // 2
                    for st in range(NQT):
                        sl = slice(st * QT, (st + 1) * QT)
                        xg = []
                        for g in range(NXG):
                            t = xtp.tile(
                                [P, XG, QT], BF16, tag="xt", name=f"xt{st}_{g}"
                            )
                            # split each piece's load so the first matmuls
                            # only wait on a sub-piece (region-level deps)
                            nsp = 4 if (st == 0 and g == 0) else 2
                            spsz = XG // nsp if XG % nsp == 0 else XG
                            for sp in range(XG // spsz):
                                nc.sync.dma_start(
                                    t[:, sp * spsz : (sp + 1) * spsz, :],
                                    xT_r[
                                        :,
                                        g * XG + sp * spsz : g * XG + (sp + 1) * spsz,
                                        sl,
                                    ],
                                )
                            xg.append(t)
                        for mb in range(hloc + 2):
                            ps = psA.tile([P, QT], F32, tag="ps", name=f"ps{st}_{mb}")
                            for kc in range(KC):
                                nc.tensor.matmul(
                                    ps,
                                    wqkv_sb[:, kc, mb * P : (mb + 1) * P],
                                    xg[kc // XG][:, kc % XG, :],
                                    start=(kc == 0),
                                    stop=(kc == KC - 1),
                                )
                            if mb <= hloc:  # q heads and k: rope
                                dst = q_sb[mb] if mb < hloc else k_sb
                                csl = cos_sb[:, sl]
                                ssl = sin_sb[:, sl]
                                t1 = ptmp.tile([H2, QT], F32, tag="t1", name=f"t1_{st}_{mb}")
                                t2 = ptmp.tile([H2, QT], F32, tag="t2", name=f"t2_{st}_{mb}")
                                nc.vector.tensor_mul(t1, ps[H2:P, :], ssl)
                                nc.vector.tensor_mul(t2, ps[0:H2, :], csl)
                                nc.vector.tensor_sub(dst[0:H2, sl], t2, t1)
                                t3 = ptmp.tile([H2, QT], F32, tag="t3", name=f"t3_{st}_{mb}")
                                t4 = ptmp.tile([H2, QT], F32, tag="t4", name=f"t4_{st}_{mb}")
                                nc.vector.tensor_mul(t3, ps[0:H2, :], ssl)
                                nc.vector.tensor_mul(t4, ps[H2:P, :], csl)
                                nc.vector.tensor_add(dst[H2:P, sl], t3, t4)
                            else:  # v: keep transposed copy, fix up below
                                nc.vector.tensor_copy(vT_sb[:, sl], ps)

                    # transpose vT -> v (natural [s, hd] layout for PV lhsT)
                    for kb in range(NKB):
                        pt = psT.tile([P, P], BF16, tag="pt", name=f"pt{kb}")
                        nc.tensor.transpose(
                            pt, vT_sb[:, kb * P : (kb + 1) * P], ident_sb
                        )
                        nc.vector.tensor_copy(v_sb[:, kb, :], pt)

                # ---------------- phase 2: attention ----------------
                with (
                    tc.tile_pool(name="psS", bufs=3, space="PSUM") as psS,
                    tc.tile_pool(name="psPA", bufs=2, space="PSUM") as psPA,
                    tc.tile_pool(name="psD", bufs=2, space="PSUM") as psD,
                ):
                    for h in range(hloc):
                        for qt in range(NQT):
                            qsl = slice(qt * QT, (qt + 1) * QT)
                            nkb = (qt + 1) * DIAG
                            pa = psPA.tile([P, QT], F32, tag="pa", name=f"pa{h}_{qt}")
                            # denominator, pre-broadcast across partitions by
                            # contracting with an all-ones [P,P] stationary
                            pd = psD.tile([P, QT], F32, tag="pd", name=f"pd{h}_{qt}")
                            for kb in range(nkb):
                                ps = psS.tile(
                                    [P, QT], F32, tag="pssc", name=f"sc{h}_{qt}_{kb}"
                                )
                                nc.tensor.matmul(
                                    ps,
                                    k_sb[:, kb * P : (kb + 1) * P],
                                    q_sb[h][:, qsl],
                                    start=True,
                                    stop=True,
                                )
                                pt = probsp.tile(
                                    [P, QT], BF16, tag="probs", name=f"pb{h}_{qt}_{kb}"
                                )
                                nc.scalar.activation(pt, ps, Exp, scale=scale)
                                o = kb - qt * DIAG
                                if o >= 0:
                                    nc.vector.tensor_mul(pt, pt, mask_sb[:, o, :])
                                nc.tensor.matmul(
                                    pa,
                                    v_sb[:, kb, :],
                                    pt,
                                    start=(kb == 0),
                                    stop=(kb == nkb - 1),
                                )
                                nc.tensor.matmul(
                                    pd,
                                    ones_bc,
                                    pt,
                                    start=(kb == 0),
                                    stop=(kb == nkb - 1),
                                )
                            den_bc = bcp.tile(
                                [P, QT], F32, tag="dbc", name=f"db{h}_{qt}"
                            )
                            nc.scalar.activation(
                                den_bc, pd, mybir.ActivationFunctionType.Copy
                            )
                            rec_bc = bcp.tile(
                                [P, QT], F32, tag="rbc", name=f"rb{h}_{qt}"
                            )
                            nc.vector.reciprocal_approx_fast(rec_bc, den_bc)
                            nc.vector.tensor_mul(
                                attnT_sb[h][:, qsl], pa, rec_bc
                            )
                        # head h complete: bounce + allgather just this head,
                        # overlapping the remaining heads' attention compute
                        nc.sync.dma_start(in_bs[h], attnT_sb[h])
                        nc.gpsimd.collective_compute(
                            "AllGather",
                            mybir.AluOpType.bypass,
                            replica_groups=[list(range(n_cores))],
                            ins=[in_bs[h].opt()],
                            outs=[ag_outs[h].opt()],
                        )

            # ---------------- phase 3: output projection ----------
            with (
                tc.tile_pool(name="agp", bufs=NDC) as agp,
                tc.tile_pool(name="wop", bufs=1) as wop,
                tc.tile_pool(name="psO", bufs=2, space="PSUM") as psO,
            ):
                wo_sb = wop.tile([P, NDC, OUTW], BF16, name="wo_sb")
                WG = max(1, NDC // 8)
                for g in range(NDC // WG):
                    nc.sync.dma_start(
                        wo_sb[:, g * WG : (g + 1) * WG, :],
                        wo_r[:, g * WG : (g + 1) * WG, :],
                    )
                # global chunk kc = rank*hloc + h lives in ag_outs[h] rows
                # [rank*P:(rank+1)*P]; load grouped by head so chunks stream
                # in as soon as that head's allgather lands
                ag_sb = [None] * NDC
                for h in range(hloc):
                    for r in range(n_cores):
                        kc = r * hloc + h
                        t = agp.tile([P, S], BF16, tag="agt", name=f"ag{kc}")
                        nc.sync.dma_start(t, ag_outs[h][r * P : (r + 1) * P, :])
                        ag_sb[kc] = t
                for sb in range(NSB):
                    po = psO.tile([P, OUTW], F32, tag="po", name=f"po{sb}")
                    for kc in range(NDC):
                        nc.tensor.matmul(
                            po,
                            ag_sb[kc][:, sb * P : (sb + 1) * P],
                            wo_sb[:, kc, :],
                            start=(kc == 0),
                            stop=(kc == NDC - 1),
                        )
                    ot = outp.tile([P, OUTW], F32, tag="ot", name=f"ot{sb}")
                    nc.vector.tensor_copy(ot, po)
                    nc.sync.dma_start(out[sb * P : (sb + 1) * P, :], ot)
    nc.finalize()
    return nc


def _rope_perm(width):
    """Per-head column permutation putting even pair-indices first."""
    blocks = []
    for h in range(width // HD):
        base = h * HD
        blocks.append(base + np.r_[0:HD:2, 1:HD:2])
    return np.concatenate(blocks)


def make_in_maps(x, wq, wk, wv, wo, cos, sin, n_cores=8, QT=512):
    """Shard + preprocess full inputs into per-core input maps."""
    bf16 = ml_dtypes.bfloat16
    S, D = x.shape[1], x.shape[2]
    nh = wq.shape[1] // HD
    hloc = nh // n_cores
    DQ = hloc * HD
    KVW = HD  # one kv head per core
    OUTW = D // n_cores
    DIAG = QT // P

    xT = np.ascontiguousarray(np.asarray(x[0]).T).astype(bf16)
    wq_p = np.asarray(wq)[:, _rope_perm(wq.shape[1])]
    wk_p = np.asarray(wk)[:, _rope_perm(wk.shape[1])]
    wv = np.asarray(wv)
    wo = np.asarray(wo)
    cosT = np.ascontiguousarray(np.asarray(cos).T).astype(np.float32)
    sinT = np.ascontiguousarray(np.asarray(sin).T).astype(np.float32)

    kk = np.arange(P)[:, None, None]
    oo = np.arange(DIAG)[None, :, None]
    qq = np.arange(QT)[None, None, :]
    maskT = ((kk + P * oo) <= qq).astype(bf16)

    in_maps = []
    for i in range(n_cores):
        wqkv_i = np.concatenate(
            [
                wq_p[:, i * DQ : (i + 1) * DQ],
                wk_p[:, i * KVW : (i + 1) * KVW],
                wv[:, i * KVW : (i + 1) * KVW],
            ],
            axis=1,
        ).astype(bf16)
        wo_i = np.ascontiguousarray(wo[:, i * OUTW : (i + 1) * OUTW]).astype(bf16)
        in_maps.append(
            {
                "xT": xT,
                "wqkv": np.ascontiguousarray(wqkv_i),
                "wo": wo_i,
                "cosT": cosT,
                "sinT": sinT,
                "maskT": np.ascontiguousarray(maskT),
            }
        )
    return in_maps


_CACHED_NC = None


def kernel(x, wq, wk, wv, wo, cos, sin):
    global _CACHED_NC
    from concourse.bass_utils import run_bass_kernel_spmd

    n_cores = N_CORES
    if _CACHED_NC is None:
        _CACHED_NC = build_nc(
            n_cores=n_cores, S=x.shape[1], D=x.shape[2], hloc=HLOC, QT=512
        )
    in_maps = make_in_maps(x, wq, wk, wv, wo, cos, sin, n_cores=n_cores, QT=512)
    res = run_bass_kernel_spmd(_CACHED_NC, in_maps, core_ids=list(range(n_cores)))
    outs = [np.asarray(res.results[i]["out"]) for i in range(n_cores)]
    full = np.concatenate(outs, axis=1).astype(np.float32)
    return full.reshape(1, x.shape[1], x.shape[2])


if __name__ == "__main__":
    nc = build_nc()
    print("graph built ok")
